# revision 3
# baseline (speedup 1.0000x reference)
"""Trainium2 Bass kernel for nn_CustomLoss_87522843558003 (YOLO CIoU+BCE loss).

Strategy (data-parallel over batch, 8 cores):
 - Each core: 8 batches; positions map to 128 SBUF partitions as
   [batch(8) x section(16)] rows of 525 positions; j-axis split into chunks
   for DMA/compute overlap (double-buffered pools).
 - Loads: SWDGE (gpsimd) cast-DMA f32->bf16, one DMA per anchor + target
   per chunk; 16-bit data halves SBUF and enables DVE 2x modes.
 - Compute rebalanced off gpsimd: DVE does the elementwise work (planar
   component-major contiguous intermediates), ACT does copies/squares/Ln
   (one table set: natural_log has Square/Copy as fillers), POOL only a few
   independent strided 2-input products.
 - Per-anchor IoU: box corners + overlap; q = I * recip_fast(I+areas-I).
   Anchor argmax = first-max cascade with copy_predicated (15-channel
   selection); the selected q IS the CIoU iou (no recompute; eps diff 1e-7).
 - arctan pair eliminated: atan(rt)-atan(rp) = atan(u) with
   u = (wt*hp - wp*ht)/(hp*ht + wt*wp); (4/pi^2)*atan^2(sqrt(z)) fitted by
   rational F(z) = z(z+b)/(z^2+cz+d).
 - BCE: obj term = -ln(p_obj) at masked positions; cls term via
   sum_c ln|p+t-1-s| = ln prod_c |...| (one reduce, one Ln); small shift s
   keeps factors nonzero under bf16 rounding (ref clamps logs anyway).
 - Per-partition masked sums via accum_out -> [128, 4*n_chunks]; host sums
   chunks and normalizes per batch.
"""

import contextlib
import numpy as np

B, A, N, CH = 64, 3, 8400, 15
NCORES = 8
BPC = B // NCORES      # batches per core
SEC = 16               # partition sections per batch
PPART = BPC * SEC      # 128 partitions
W5 = N // SEC          # 525 positions per partition row
NCLS = 10
EPS = 1e-7
# rational fit of (4/pi^2)*atan(sqrt(z))^2
FB = 18.5807497
FC = 29.74781457
FD = 47.19260109

N_CHUNKS = 3
HALF = "bf16"

_CACHE = {}


def _build_bass(loop_r=None, n_chunks=N_CHUNKS, half=HALF, level=4,
                sbufs=3):
    import concourse.tile as tile
    import concourse.mybir as mybir
    from concourse import bacc

    Alu = mybir.AluOpType
    Act = mybir.ActivationFunctionType
    f32 = mybir.dt.float32
    i32 = mybir.dt.int32
    if half == "bf16":
        dtl, i16 = mybir.dt.bfloat16, mybir.dt.int16
    elif half == "fp16":
        dtl, i16 = mybir.dt.float16, mybir.dt.int16
    else:
        dtl, i16 = f32, i32
    cast = dtl != f32

    nc = bacc.Bacc("TRN2", target_bir_lowering=False, debug=False,
                   num_devices=NCORES)
    predL = nc.dram_tensor("predL", [BPC, A, N, CH], f32, kind="ExternalInput").ap()
    targL = nc.dram_tensor("targL", [BPC, N, CH], f32, kind="ExternalInput").ap()
    NACC = 4 * n_chunks
    accO = nc.dram_tensor("acc_out", [PPART, NACC], f32, kind="ExternalOutput").ap()

    base = W5 // n_chunks
    sizes = [base + (1 if i < W5 % n_chunks else 0) for i in range(n_chunks)]
    offs = [sum(sizes[:i]) for i in range(n_chunks)]

    pre = predL.rearrange("b a (s j) c -> b a s (j c)", s=SEC)
    tre = targL.rearrange("b (s j) c -> b s (j c)", s=SEC)

    with tile.TileContext(nc) as tc:
        with (
            tc.tile_pool(name="pIO", bufs=2) as pIO,
            tc.tile_pool(name="pS", bufs=sbufs) as pS,
            tc.tile_pool(name="pAcc", bufs=1) as pAcc,
        ):
            ACC = pAcc.tile([PPART, NACC], f32)
            loop_cm = tc.For_i(0, loop_r, 1) if loop_r else contextlib.nullcontext()
            with loop_cm:
                for k in range(n_chunks):
                    W, ofs = sizes[k], offs[k]
                    lo, hi = ofs * CH, (ofs + W) * CH

                    # ---- loads ----
                    T = pIO.tile([PPART, W * CH], dtl, name="T")
                    Ps = []
                    if cast:
                        nc.gpsimd.dma_start(T[:], tre[:, :, lo:hi])
                    else:
                        nc.scalar.dma_start(T[:], tre[:, :, lo:hi])
                    rings = [nc.sync, nc.scalar, nc.sync]
                    for a in range(A):
                        P = pIO.tile([PPART, W * CH], dtl, name=f"P{a}")
                        if cast:
                            nc.gpsimd.dma_start(P[:], pre[:, a][:, :, lo:hi])
                        else:
                            rings[a].dma_start(P[:], pre[:, a][:, :, lo:hi])
                        Ps.append(P)

                    DUM = pS.tile([PPART, W], f32, name="DUM")
                    if level == 0:
                        nc.vector.tensor_scalar(
                            DUM[:], T[:, 0:W], 1.0, 0.0, Alu.mult, Alu.add,
                            accum_out=ACC[:, 4 * k:4 * k + 1])
                        for a in range(A):
                            nc.vector.tensor_scalar(
                                DUM[:], Ps[a][:, 0:W], 1.0, 0.0, Alu.mult,
                                Alu.add,
                                accum_out=ACC[:, 4 * k + 1 + a:4 * k + 2 + a])
                        continue

                    Tcj = T[:].rearrange("p (j c) -> p c j", c=CH)
                    Tr = T[:].rearrange("p (j c) -> p j c", c=CH)

                    # ---- target prep ----
                    TLO = pS.tile([PPART, 2 * W], dtl, name="TLO")
                    THI = pS.tile([PPART, 2 * W], dtl, name="THI")
                    TA = pS.tile([PPART, W], dtl, name="TA")
                    MSK = pS.tile([PPART, W], f32, name="MSK")
                    tlov = TLO[:].rearrange("p (c j) -> p c j", c=2)
                    thiv = THI[:].rearrange("p (c j) -> p c j", c=2)
                    nc.vector.scalar_tensor_tensor(
                        tlov, Tcj[:, 2:4], -0.5, Tcj[:, 0:2], Alu.mult, Alu.add)
                    nc.vector.scalar_tensor_tensor(
                        thiv, Tcj[:, 2:4], 0.5, Tcj[:, 0:2], Alu.mult, Alu.add)
                    nc.gpsimd.tensor_tensor(TA[:], Tcj[:, 2], Tcj[:, 3], Alu.mult)
                    nc.scalar.activation(MSK[:], Tcj[:, 4], Act.Copy)

                    # ---- per-anchor IoU (I, U, q = I/U) ----
                    Qs = []
                    for a in range(A):
                        Pcj = Ps[a][:].rearrange("p (j c) -> p c j", c=CH)
                        WHH = pS.tile([PPART, 2 * W], dtl, name=f"WHH{a}")
                        LO = pS.tile([PPART, 2 * W], dtl, name=f"LO{a}")
                        HI = pS.tile([PPART, 2 * W], dtl, name=f"HI{a}")
                        WD = pS.tile([PPART, 2 * W], dtl, name=f"WD{a}")
                        whhv = WHH[:].rearrange("p (c j) -> p c j", c=2)
                        lov = LO[:].rearrange("p (c j) -> p c j", c=2)
                        hiv = HI[:].rearrange("p (c j) -> p c j", c=2)
                        nc.scalar.activation(whhv, Pcj[:, 2:4], Act.Copy,
                                             scale=0.5)
                        nc.vector.tensor_tensor(lov, Pcj[:, 0:2], whhv,
                                                Alu.subtract)
                        nc.vector.tensor_tensor(hiv, Pcj[:, 0:2], whhv,
                                                Alu.add)
                        nc.vector.tensor_tensor(WHH[:], LO[:], TLO[:], Alu.max)
                        nc.vector.tensor_tensor(WD[:], HI[:], THI[:], Alu.min)
                        nc.vector.tensor_tensor(WD[:], WD[:], WHH[:],
                                                Alu.subtract)
                        nc.vector.tensor_scalar(WD[:], WD[:], 0.0, None, Alu.max)
                        IA = pS.tile([PPART, W], dtl, name=f"IA{a}")
                        PAr = pS.tile([PPART, W], dtl, name=f"PAr{a}")
                        UA = pS.tile([PPART, W], f32, name=f"UA{a}")
                        QA = pS.tile([PPART, W], f32, name=f"QA{a}")
                        nc.vector.tensor_tensor(IA[:], WD[:, 0:W], WD[:, W:],
                                                Alu.mult)
                        nc.gpsimd.tensor_tensor(PAr[:], Pcj[:, 2], Pcj[:, 3],
                                                Alu.mult)
                        nc.vector.tensor_tensor(UA[:], PAr[:], TA[:], Alu.add)
                        nc.vector.tensor_tensor(UA[:], UA[:], IA[:],
                                                Alu.subtract)
                        nc.vector.reciprocal_approx_fast(QA[:], UA[:])
                        nc.vector.tensor_tensor(QA[:], IA[:], QA[:], Alu.mult)
                        Qs.append(QA)
                        if level == 1:
                            nc.vector.tensor_scalar(
                                DUM[:], QA[:], 1.0, 0.0, Alu.mult, Alu.add,
                                accum_out=ACC[:, 4 * k + a:4 * k + a + 1])
                    if level == 1:
                        continue

                    # ---- selection cascade (first-max argmax) ----
                    SELP = Ps[0]
                    SELQ = Qs[0]
                    for a in (1, 2):
                        G = pS.tile([PPART, W], f32, name=f"G{a}")
                        nc.vector.tensor_tensor(G[:], SELQ[:], Qs[a][:],
                                                Alu.is_ge)
                        nc.vector.copy_predicated(
                            Qs[a][:], G[:].bitcast(i32), SELQ[:])
                        if cast:
                            GH = pS.tile([PPART, W], dtl, name=f"GH{a}")
                            nc.vector.tensor_copy(GH[:], G[:])
                            mask = GH[:].bitcast(i16).unsqueeze(2).broadcast_to(
                                [PPART, W, CH])
                        else:
                            mask = G[:].bitcast(i32).unsqueeze(2).broadcast_to(
                                [PPART, W, CH])
                        nc.vector.copy_predicated(
                            Ps[a][:].rearrange("p (j c) -> p j c", c=CH), mask,
                            SELP[:].rearrange("p (j c) -> p j c", c=CH))
                        SELP = Ps[a]
                        SELQ = Qs[a]
                    IOU = SELQ  # f32 [P, W] == selected inter/union
                    if level == 2:
                        nc.vector.tensor_scalar(
                            DUM[:], IOU[:], 1.0, 0.0, Alu.mult, Alu.add,
                            accum_out=ACC[:, 4 * k:4 * k + 1])
                        nc.vector.tensor_scalar(
                            DUM[:], SELP[:, 0:W], 1.0, 0.0, Alu.mult, Alu.add,
                            accum_out=ACC[:, 4 * k + 1:4 * k + 2])
                        continue

                    Scj = SELP[:].rearrange("p (j c) -> p c j", c=CH)
                    Sr = SELP[:].rearrange("p (j c) -> p j c", c=CH)

                    # ---- CIoU on selected ----
                    SWH = pS.tile([PPART, 2 * W], dtl, name="SWH")
                    SLO = pS.tile([PPART, 2 * W], dtl, name="SLO")
                    SHI = pS.tile([PPART, 2 * W], dtl, name="SHI")
                    CW = pS.tile([PPART, 2 * W], dtl, name="CW")
                    CW2 = pS.tile([PPART, 2 * W], f32, name="CW2")
                    swhv = SWH[:].rearrange("p (c j) -> p c j", c=2)
                    slov = SLO[:].rearrange("p (c j) -> p c j", c=2)
                    shiv = SHI[:].rearrange("p (c j) -> p c j", c=2)
                    nc.scalar.activation(swhv, Scj[:, 2:4], Act.Copy, scale=0.5)
                    nc.vector.tensor_tensor(slov, Scj[:, 0:2], swhv,
                                            Alu.subtract)
                    nc.vector.tensor_tensor(shiv, Scj[:, 0:2], swhv, Alu.add)
                    nc.vector.tensor_tensor(SLO[:], SLO[:], TLO[:], Alu.min)
                    nc.vector.tensor_tensor(SHI[:], SHI[:], THI[:], Alu.max)
                    nc.vector.tensor_tensor(CW[:], SHI[:], SLO[:], Alu.subtract)
                    nc.scalar.activation(CW2[:], CW[:], Act.Square)
                    DIAG = pS.tile([PPART, W], f32, name="DIAG")
                    RDG = pS.tile([PPART, W], f32, name="RDG")
                    nc.vector.scalar_tensor_tensor(
                        DIAG[:], CW2[:, 0:W], EPS, CW2[:, W:], Alu.add, Alu.add)
                    nc.vector.reciprocal_approx_fast(RDG[:], DIAG[:])
                    DXY = pS.tile([PPART, 2 * W], dtl, name="DXY")
                    DXY2 = pS.tile([PPART, 2 * W], f32, name="DXY2")
                    nc.vector.tensor_tensor(
                        DXY[:].rearrange("p (c j) -> p c j", c=2),
                        Scj[:, 0:2], Tcj[:, 0:2], Alu.subtract)
                    nc.scalar.activation(DXY2[:], DXY[:], Act.Square)
                    CD = pS.tile([PPART, W], f32, name="CD")
                    QD = pS.tile([PPART, W], f32, name="QD")
                    OMIE = pS.tile([PPART, W], f32, name="OMIE")
                    DIOU = pS.tile([PPART, W], f32, name="DIOU")
                    nc.gpsimd.tensor_tensor(CD[:], DXY2[:, 0:W], DXY2[:, W:],
                                            Alu.add)
                    nc.vector.tensor_tensor(QD[:], CD[:], RDG[:], Alu.mult)
                    nc.vector.tensor_scalar(OMIE[:], IOU[:], -1.0, 1.0 + EPS,
                                            Alu.mult, Alu.add)
                    nc.gpsimd.tensor_tensor(DIOU[:], OMIE[:], QD[:], Alu.add)

                    # v-term
                    N1 = pS.tile([PPART, W], dtl, name="N1")
                    N2 = pS.tile([PPART, W], dtl, name="N2")
                    D1 = pS.tile([PPART, W], dtl, name="D1")
                    D2 = pS.tile([PPART, W], dtl, name="D2")
                    nc.gpsimd.tensor_tensor(N1[:], Tcj[:, 2], Scj[:, 3], Alu.mult)
                    nc.gpsimd.tensor_tensor(N2[:], Scj[:, 2], Tcj[:, 3], Alu.mult)
                    nc.gpsimd.tensor_tensor(D1[:], Scj[:, 3], Tcj[:, 3], Alu.mult)
                    nc.gpsimd.tensor_tensor(D2[:], Scj[:, 2], Tcj[:, 2], Alu.mult)
                    NUM = pS.tile([PPART, W], f32, name="NUM")
                    DEN = pS.tile([PPART, W], f32, name="DEN")
                    UU = pS.tile([PPART, W], f32, name="UU")
                    ZZ = pS.tile([PPART, W], f32, name="ZZ")
                    VN = pS.tile([PPART, W], f32, name="VN")
                    VD = pS.tile([PPART, W], f32, name="VD")
                    VV = pS.tile([PPART, W], f32, name="VV")
                    AD = pS.tile([PPART, W], f32, name="AD")
                    CIO = pS.tile([PPART, W], f32, name="CIO")
                    nc.gpsimd.tensor_tensor(NUM[:], N1[:], N2[:], Alu.subtract)
                    nc.gpsimd.tensor_tensor(DEN[:], D1[:], D2[:], Alu.add)
                    nc.vector.reciprocal_approx_fast(UU[:], DEN[:])
                    nc.vector.tensor_tensor(UU[:], NUM[:], UU[:], Alu.mult)
                    nc.scalar.activation(ZZ[:], UU[:], Act.Square)
                    nc.vector.scalar_tensor_tensor(VN[:], ZZ[:], FB, ZZ[:],
                                                   Alu.add, Alu.mult)
                    nc.vector.scalar_tensor_tensor(VD[:], ZZ[:], FC, ZZ[:],
                                                   Alu.add, Alu.mult)
                    nc.vector.tensor_scalar(VD[:], VD[:], 1.0, FD, Alu.mult,
                                            Alu.add)
                    nc.vector.reciprocal_approx_fast(VV[:], VD[:])
                    nc.vector.tensor_tensor(VV[:], VN[:], VV[:], Alu.mult)
                    nc.vector.tensor_tensor(AD[:], VV[:], OMIE[:], Alu.add)
                    nc.vector.reciprocal_approx_fast(AD[:], AD[:])
                    nc.scalar.activation(VN[:], VV[:], Act.Square)
                    nc.vector.tensor_tensor(AD[:], VN[:], AD[:], Alu.mult)
                    nc.vector.tensor_tensor(CIO[:], DIOU[:], AD[:], Alu.add)
                    if level == 3:
                        nc.vector.scalar_tensor_tensor(
                            DUM[:], CIO[:], 1.0, MSK[:], Alu.mult, Alu.mult,
                            accum_out=ACC[:, 4 * k:4 * k + 1])
                        nc.vector.tensor_scalar(
                            DUM[:], MSK[:], 1.0, 0.0, Alu.mult, Alu.add,
                            accum_out=ACC[:, 4 * k + 3:4 * k + 4])
                        continue

                    # ---- BCE prep ----
                    LNIN = pS.tile([PPART, 2 * W], f32, name="LNIN")
                    LNO = pS.tile([PPART, 2 * W], f32, name="LNO")
                    DT = pS.tile([PPART, W * NCLS], dtl, name="DT")
                    nc.scalar.activation(LNIN[:, 0:W], Scj[:, 4], Act.Copy)
                    # shift keeps |p+t-1-shift| > 0 under 16-bit rounding
                    # (ref clamps logs at -100 anyway); bias ~4e-3 per term
                    shift = -1.0005 if cast else -1.0
                    nc.vector.scalar_tensor_tensor(
                        DT[:].rearrange("p (j c) -> p j c", c=NCLS),
                        Sr[:, :, 5:CH], shift, Tr[:, :, 5:CH], Alu.add, Alu.add)
                    nc.vector.tensor_reduce(
                        LNIN[:, W:2 * W],
                        DT[:].rearrange("p (j c) -> p j c", c=NCLS),
                        mybir.AxisListType.X, Alu.mult,
                        apply_absolute_value=True)
                    nc.scalar.activation(LNO[:], LNIN[:], Act.Ln)

                    # ---- masked accums ----
                    nc.vector.scalar_tensor_tensor(
                        DUM[:], CIO[:], 1.0, MSK[:], Alu.mult, Alu.mult,
                        accum_out=ACC[:, 4 * k:4 * k + 1])
                    nc.vector.scalar_tensor_tensor(
                        DUM[:], LNO[:, 0:W], 1.0, MSK[:], Alu.mult, Alu.mult,
                        accum_out=ACC[:, 4 * k + 1:4 * k + 2])
                    nc.vector.scalar_tensor_tensor(
                        DUM[:], LNO[:, W:2 * W], 1.0, MSK[:], Alu.mult,
                        Alu.mult, accum_out=ACC[:, 4 * k + 2:4 * k + 3])
                    nc.vector.tensor_scalar(
                        DUM[:], MSK[:], 1.0, 0.0, Alu.mult, Alu.add,
                        accum_out=ACC[:, 4 * k + 3:4 * k + 4])

            nc.sync.dma_start(accO, ACC[:])

    nc.compile()
    return nc


def kernel(pred, target):
    pred = np.ascontiguousarray(np.asarray(pred, dtype=np.float32))
    target = np.ascontiguousarray(np.asarray(target, dtype=np.float32))
    assert pred.shape == (B, A, N, CH) and target.shape == (B, N, CH)

    if "nc" not in _CACHE:
        _CACHE["nc"] = _build_bass()
    nc = _CACHE["nc"]

    from concourse import bass_utils

    in_maps = []
    for c in range(NCORES):
        lo, hi = c * BPC, (c + 1) * BPC
        in_maps.append({
            "predL": np.ascontiguousarray(pred[lo:hi]),
            "targL": np.ascontiguousarray(target[lo:hi]),
        })

    res = None
    for attempt in range(3):
        try:
            res = bass_utils.run_bass_kernel_spmd(
                nc, in_maps, core_ids=list(range(NCORES)))
            break
        except Exception:
            if attempt == 2:
                raise
    _CACHE["last_results"] = res

    per_batch = []
    for c in range(NCORES):
        acc = res.results[c]["acc_out"].astype(np.float32)
        acc = acc.reshape(PPART, N_CHUNKS, 4).sum(axis=1)
        num = acc[:, 0] - acc[:, 1] - 0.1 * acc[:, 2]
        cnt = acc[:, 3]
        nb = num.reshape(BPC, SEC).sum(axis=1, dtype=np.float32)
        cb = cnt.reshape(BPC, SEC).sum(axis=1, dtype=np.float32)
        per_batch.append(nb / cb)
    loss = np.mean(np.concatenate(per_batch), dtype=np.float32)
    return np.float32(loss)


# revision 4
# speedup vs baseline: 1.8719x; 1.8719x over previous
"""Trainium2 Bass kernel for nn_CustomLoss_87522843558003 (YOLO CIoU+BCE loss).

Strategy (data-parallel over batch, 8 cores):
 - Each core: 8 batches; positions map to 128 SBUF partitions as
   [batch(8) x section(16)] rows of 525 positions; j-axis split into chunks
   for DMA/compute overlap (double-buffered pools).
 - Loads: SWDGE (gpsimd) cast-DMA f32->bf16, one DMA per anchor + target
   per chunk; 16-bit data halves SBUF and enables DVE 2x modes.
 - Compute rebalanced off gpsimd: DVE does the elementwise work (planar
   component-major contiguous intermediates), ACT does copies/squares/Ln
   (one table set: natural_log has Square/Copy as fillers), POOL only a few
   independent strided 2-input products.
 - Per-anchor IoU: box corners + overlap; q = I * recip_fast(I+areas-I).
   Anchor argmax = first-max cascade with copy_predicated (15-channel
   selection); the selected q IS the CIoU iou (no recompute; eps diff 1e-7).
 - arctan pair eliminated: atan(rt)-atan(rp) = atan(u) with
   u = (wt*hp - wp*ht)/(hp*ht + wt*wp); (4/pi^2)*atan^2(sqrt(z)) fitted by
   rational F(z) = z(z+b)/(z^2+cz+d).
 - BCE: obj term = -ln(p_obj) at masked positions; cls term via
   sum_c ln|p+t-1-s| = ln prod_c |...| (one reduce, one Ln); small shift s
   keeps factors nonzero under bf16 rounding (ref clamps logs anyway).
 - Per-partition masked sums via accum_out -> [128, 4*n_chunks]; host sums
   chunks and normalizes per batch.
"""

import contextlib
import numpy as np

B, A, N, CH = 64, 3, 8400, 15
NCORES = 8
BPC = B // NCORES      # batches per core
SEC = 16               # partition sections per batch
PPART = BPC * SEC      # 128 partitions
W5 = N // SEC          # 525 positions per partition row
NCLS = 10
EPS = 1e-7
# rational fit of (4/pi^2)*atan(sqrt(z))^2
FB = 18.5807497
FC = 29.74781457
FD = 47.19260109

N_CHUNKS = 2
HALF = "bf16"

_CACHE = {}


def _build_bass(loop_r=None, n_chunks=N_CHUNKS, half=HALF, level=4,
                sbufs=2):
    import concourse.tile as tile
    import concourse.mybir as mybir
    from concourse import bacc

    Alu = mybir.AluOpType
    Act = mybir.ActivationFunctionType
    f32 = mybir.dt.float32
    i32 = mybir.dt.int32
    if half == "bf16":
        dtl, i16 = mybir.dt.bfloat16, mybir.dt.int16
    elif half == "fp16":
        dtl, i16 = mybir.dt.float16, mybir.dt.int16
    else:
        dtl, i16 = f32, i32
    cast = dtl != f32

    nc = bacc.Bacc("TRN2", target_bir_lowering=False, debug=False,
                   num_devices=NCORES)
    predL = nc.dram_tensor("predL", [BPC, A, N, CH], f32, kind="ExternalInput").ap()
    targL = nc.dram_tensor("targL", [BPC, N, CH], f32, kind="ExternalInput").ap()
    NACC = 4 * n_chunks
    accO = nc.dram_tensor("acc_out", [PPART, NACC], f32, kind="ExternalOutput").ap()

    base = W5 // n_chunks
    sizes = [base + (1 if i < W5 % n_chunks else 0) for i in range(n_chunks)]
    offs = [sum(sizes[:i]) for i in range(n_chunks)]

    pre = predL.rearrange("b a (s j) c -> b a s (j c)", s=SEC)
    tre = targL.rearrange("b (s j) c -> b s (j c)", s=SEC)

    with tile.TileContext(nc) as tc:
        with (
            tc.tile_pool(name="pIO", bufs=2) as pIO,
            tc.tile_pool(name="pS", bufs=sbufs) as pS,
            tc.tile_pool(name="pAcc", bufs=1) as pAcc,
        ):
            ACC = pAcc.tile([PPART, NACC], f32)
            loop_cm = tc.For_i(0, loop_r, 1) if loop_r else contextlib.nullcontext()
            with loop_cm:
                for k in range(n_chunks):
                    W, ofs = sizes[k], offs[k]
                    lo, hi = ofs * CH, (ofs + W) * CH

                    # ---- loads ----
                    T = pIO.tile([PPART, W * CH], dtl, name="T")
                    Ps = []
                    if cast:
                        nc.gpsimd.dma_start(T[:], tre[:, :, lo:hi])
                    else:
                        nc.scalar.dma_start(T[:], tre[:, :, lo:hi])
                    rings = [nc.sync, nc.scalar, nc.sync]
                    for a in range(A):
                        P = pIO.tile([PPART, W * CH], dtl, name=f"P{a}")
                        if cast:
                            nc.gpsimd.dma_start(P[:], pre[:, a][:, :, lo:hi])
                        else:
                            rings[a].dma_start(P[:], pre[:, a][:, :, lo:hi])
                        Ps.append(P)

                    DUM = pS.tile([PPART, W], f32, name="DUM")
                    if level == 0:
                        nc.vector.tensor_scalar(
                            DUM[:], T[:, 0:W], 1.0, 0.0, Alu.mult, Alu.add,
                            accum_out=ACC[:, 4 * k:4 * k + 1])
                        for a in range(A):
                            nc.vector.tensor_scalar(
                                DUM[:], Ps[a][:, 0:W], 1.0, 0.0, Alu.mult,
                                Alu.add,
                                accum_out=ACC[:, 4 * k + 1 + a:4 * k + 2 + a])
                        continue

                    Tcj = T[:].rearrange("p (j c) -> p c j", c=CH)
                    Tr = T[:].rearrange("p (j c) -> p j c", c=CH)

                    # ---- target prep ----
                    TLO = pS.tile([PPART, 2 * W], dtl, name="TLO")
                    THI = pS.tile([PPART, 2 * W], dtl, name="THI")
                    TA = pS.tile([PPART, W], dtl, name="TA")
                    MSK = pS.tile([PPART, W], f32, name="MSK")
                    tlov = TLO[:].rearrange("p (c j) -> p c j", c=2)
                    thiv = THI[:].rearrange("p (c j) -> p c j", c=2)
                    nc.vector.scalar_tensor_tensor(
                        tlov, Tcj[:, 2:4], -0.5, Tcj[:, 0:2], Alu.mult, Alu.add)
                    nc.vector.scalar_tensor_tensor(
                        thiv, Tcj[:, 2:4], 0.5, Tcj[:, 0:2], Alu.mult, Alu.add)
                    nc.gpsimd.tensor_tensor(TA[:], Tcj[:, 2], Tcj[:, 3], Alu.mult)
                    nc.scalar.activation(MSK[:], Tcj[:, 4], Act.Copy)

                    # ---- per-anchor IoU (I, U, q = I/U) ----
                    Qs = []
                    for a in range(A):
                        Pcj = Ps[a][:].rearrange("p (j c) -> p c j", c=CH)
                        WHH = pS.tile([PPART, 2 * W], dtl, name=f"WHH{a}")
                        LO = pS.tile([PPART, 2 * W], dtl, name=f"LO{a}")
                        HI = pS.tile([PPART, 2 * W], dtl, name=f"HI{a}")
                        WD = pS.tile([PPART, 2 * W], dtl, name=f"WD{a}")
                        whhv = WHH[:].rearrange("p (c j) -> p c j", c=2)
                        lov = LO[:].rearrange("p (c j) -> p c j", c=2)
                        hiv = HI[:].rearrange("p (c j) -> p c j", c=2)
                        nc.scalar.activation(whhv, Pcj[:, 2:4], Act.Copy,
                                             scale=0.5)
                        nc.vector.tensor_tensor(lov, Pcj[:, 0:2], whhv,
                                                Alu.subtract)
                        nc.vector.tensor_tensor(hiv, Pcj[:, 0:2], whhv,
                                                Alu.add)
                        nc.vector.tensor_tensor(WHH[:], LO[:], TLO[:], Alu.max)
                        nc.vector.tensor_tensor(WD[:], HI[:], THI[:], Alu.min)
                        nc.vector.tensor_tensor(WD[:], WD[:], WHH[:],
                                                Alu.subtract)
                        nc.vector.tensor_scalar(WD[:], WD[:], 0.0, None, Alu.max)
                        IA = pS.tile([PPART, W], dtl, name=f"IA{a}")
                        PAr = pS.tile([PPART, W], dtl, name=f"PAr{a}")
                        UA = pS.tile([PPART, W], f32, name=f"UA{a}")
                        QA = pS.tile([PPART, W], f32, name=f"QA{a}")
                        nc.vector.tensor_tensor(IA[:], WD[:, 0:W], WD[:, W:],
                                                Alu.mult)
                        nc.gpsimd.tensor_tensor(PAr[:], Pcj[:, 2], Pcj[:, 3],
                                                Alu.mult)
                        nc.vector.tensor_tensor(UA[:], PAr[:], TA[:], Alu.add)
                        nc.vector.tensor_tensor(UA[:], UA[:], IA[:],
                                                Alu.subtract)
                        nc.vector.reciprocal_approx_fast(QA[:], UA[:])
                        nc.vector.tensor_tensor(QA[:], IA[:], QA[:], Alu.mult)
                        Qs.append(QA)
                        if level == 1:
                            nc.vector.tensor_scalar(
                                DUM[:], QA[:], 1.0, 0.0, Alu.mult, Alu.add,
                                accum_out=ACC[:, 4 * k + a:4 * k + a + 1])
                    if level == 1:
                        continue

                    # ---- selection cascade (first-max argmax) ----
                    SELP = Ps[0]
                    SELQ = Qs[0]
                    for a in (1, 2):
                        G = pS.tile([PPART, W], f32, name=f"G{a}")
                        nc.vector.tensor_tensor(G[:], SELQ[:], Qs[a][:],
                                                Alu.is_ge)
                        nc.vector.copy_predicated(
                            Qs[a][:], G[:].bitcast(i32), SELQ[:])
                        if cast:
                            GH = pS.tile([PPART, W], dtl, name=f"GH{a}")
                            nc.vector.tensor_copy(GH[:], G[:])
                            mask = GH[:].bitcast(i16).unsqueeze(2).broadcast_to(
                                [PPART, W, CH])
                        else:
                            mask = G[:].bitcast(i32).unsqueeze(2).broadcast_to(
                                [PPART, W, CH])
                        nc.vector.copy_predicated(
                            Ps[a][:].rearrange("p (j c) -> p j c", c=CH), mask,
                            SELP[:].rearrange("p (j c) -> p j c", c=CH))
                        SELP = Ps[a]
                        SELQ = Qs[a]
                    IOU = SELQ  # f32 [P, W] == selected inter/union
                    if level == 2:
                        nc.vector.tensor_scalar(
                            DUM[:], IOU[:], 1.0, 0.0, Alu.mult, Alu.add,
                            accum_out=ACC[:, 4 * k:4 * k + 1])
                        nc.vector.tensor_scalar(
                            DUM[:], SELP[:, 0:W], 1.0, 0.0, Alu.mult, Alu.add,
                            accum_out=ACC[:, 4 * k + 1:4 * k + 2])
                        continue

                    Scj = SELP[:].rearrange("p (j c) -> p c j", c=CH)
                    Sr = SELP[:].rearrange("p (j c) -> p j c", c=CH)

                    # ---- CIoU on selected ----
                    SWH = pS.tile([PPART, 2 * W], dtl, name="SWH")
                    SLO = pS.tile([PPART, 2 * W], dtl, name="SLO")
                    SHI = pS.tile([PPART, 2 * W], dtl, name="SHI")
                    CW = pS.tile([PPART, 2 * W], dtl, name="CW")
                    CW2 = pS.tile([PPART, 2 * W], f32, name="CW2")
                    swhv = SWH[:].rearrange("p (c j) -> p c j", c=2)
                    slov = SLO[:].rearrange("p (c j) -> p c j", c=2)
                    shiv = SHI[:].rearrange("p (c j) -> p c j", c=2)
                    nc.scalar.activation(swhv, Scj[:, 2:4], Act.Copy, scale=0.5)
                    nc.vector.tensor_tensor(slov, Scj[:, 0:2], swhv,
                                            Alu.subtract)
                    nc.vector.tensor_tensor(shiv, Scj[:, 0:2], swhv, Alu.add)
                    nc.vector.tensor_tensor(SLO[:], SLO[:], TLO[:], Alu.min)
                    nc.vector.tensor_tensor(SHI[:], SHI[:], THI[:], Alu.max)
                    nc.vector.tensor_tensor(CW[:], SHI[:], SLO[:], Alu.subtract)
                    nc.scalar.activation(CW2[:], CW[:], Act.Square)
                    DIAG = pS.tile([PPART, W], f32, name="DIAG")
                    RDG = pS.tile([PPART, W], f32, name="RDG")
                    nc.vector.scalar_tensor_tensor(
                        DIAG[:], CW2[:, 0:W], EPS, CW2[:, W:], Alu.add, Alu.add)
                    nc.vector.reciprocal_approx_fast(RDG[:], DIAG[:])
                    DXY = pS.tile([PPART, 2 * W], dtl, name="DXY")
                    DXY2 = pS.tile([PPART, 2 * W], f32, name="DXY2")
                    nc.vector.tensor_tensor(
                        DXY[:].rearrange("p (c j) -> p c j", c=2),
                        Scj[:, 0:2], Tcj[:, 0:2], Alu.subtract)
                    nc.scalar.activation(DXY2[:], DXY[:], Act.Square)
                    CD = pS.tile([PPART, W], f32, name="CD")
                    QD = pS.tile([PPART, W], f32, name="QD")
                    OMIE = pS.tile([PPART, W], f32, name="OMIE")
                    DIOU = pS.tile([PPART, W], f32, name="DIOU")
                    nc.gpsimd.tensor_tensor(CD[:], DXY2[:, 0:W], DXY2[:, W:],
                                            Alu.add)
                    nc.vector.tensor_tensor(QD[:], CD[:], RDG[:], Alu.mult)
                    nc.vector.tensor_scalar(OMIE[:], IOU[:], -1.0, 1.0 + EPS,
                                            Alu.mult, Alu.add)
                    nc.gpsimd.tensor_tensor(DIOU[:], OMIE[:], QD[:], Alu.add)

                    # v-term
                    N1 = pS.tile([PPART, W], dtl, name="N1")
                    N2 = pS.tile([PPART, W], dtl, name="N2")
                    D1 = pS.tile([PPART, W], dtl, name="D1")
                    D2 = pS.tile([PPART, W], dtl, name="D2")
                    nc.gpsimd.tensor_tensor(N1[:], Tcj[:, 2], Scj[:, 3], Alu.mult)
                    nc.gpsimd.tensor_tensor(N2[:], Scj[:, 2], Tcj[:, 3], Alu.mult)
                    nc.gpsimd.tensor_tensor(D1[:], Scj[:, 3], Tcj[:, 3], Alu.mult)
                    nc.gpsimd.tensor_tensor(D2[:], Scj[:, 2], Tcj[:, 2], Alu.mult)
                    NUM = pS.tile([PPART, W], f32, name="NUM")
                    DEN = pS.tile([PPART, W], f32, name="DEN")
                    UU = pS.tile([PPART, W], f32, name="UU")
                    ZZ = pS.tile([PPART, W], f32, name="ZZ")
                    VN = pS.tile([PPART, W], f32, name="VN")
                    VD = pS.tile([PPART, W], f32, name="VD")
                    VV = pS.tile([PPART, W], f32, name="VV")
                    AD = pS.tile([PPART, W], f32, name="AD")
                    CIO = pS.tile([PPART, W], f32, name="CIO")
                    nc.gpsimd.tensor_tensor(NUM[:], N1[:], N2[:], Alu.subtract)
                    nc.gpsimd.tensor_tensor(DEN[:], D1[:], D2[:], Alu.add)
                    nc.vector.reciprocal_approx_fast(UU[:], DEN[:])
                    nc.vector.tensor_tensor(UU[:], NUM[:], UU[:], Alu.mult)
                    nc.scalar.activation(ZZ[:], UU[:], Act.Square)
                    nc.vector.scalar_tensor_tensor(VN[:], ZZ[:], FB, ZZ[:],
                                                   Alu.add, Alu.mult)
                    nc.vector.scalar_tensor_tensor(VD[:], ZZ[:], FC, ZZ[:],
                                                   Alu.add, Alu.mult)
                    nc.vector.tensor_scalar(VD[:], VD[:], 1.0, FD, Alu.mult,
                                            Alu.add)
                    nc.vector.reciprocal_approx_fast(VV[:], VD[:])
                    nc.vector.tensor_tensor(VV[:], VN[:], VV[:], Alu.mult)
                    nc.vector.tensor_tensor(AD[:], VV[:], OMIE[:], Alu.add)
                    nc.vector.reciprocal_approx_fast(AD[:], AD[:])
                    nc.scalar.activation(VN[:], VV[:], Act.Square)
                    nc.vector.tensor_tensor(AD[:], VN[:], AD[:], Alu.mult)
                    nc.vector.tensor_tensor(CIO[:], DIOU[:], AD[:], Alu.add)
                    if level == 3:
                        nc.vector.scalar_tensor_tensor(
                            DUM[:], CIO[:], 1.0, MSK[:], Alu.mult, Alu.mult,
                            accum_out=ACC[:, 4 * k:4 * k + 1])
                        nc.vector.tensor_scalar(
                            DUM[:], MSK[:], 1.0, 0.0, Alu.mult, Alu.add,
                            accum_out=ACC[:, 4 * k + 3:4 * k + 4])
                        continue

                    # ---- BCE prep ----
                    LNIN = pS.tile([PPART, 2 * W], f32, name="LNIN")
                    LNO = pS.tile([PPART, 2 * W], f32, name="LNO")
                    DT = pS.tile([PPART, W * NCLS], dtl, name="DT")
                    nc.scalar.activation(LNIN[:, 0:W], Scj[:, 4], Act.Copy)
                    # shift keeps |p+t-1-shift| > 0 under 16-bit rounding
                    # (ref clamps logs at -100 anyway); bias ~4e-3 per term
                    shift = -1.0005 if cast else -1.0
                    nc.vector.scalar_tensor_tensor(
                        DT[:].rearrange("p (j c) -> p j c", c=NCLS),
                        Sr[:, :, 5:CH], shift, Tr[:, :, 5:CH], Alu.add, Alu.add)
                    nc.vector.tensor_reduce(
                        LNIN[:, W:2 * W],
                        DT[:].rearrange("p (j c) -> p j c", c=NCLS),
                        mybir.AxisListType.X, Alu.mult,
                        apply_absolute_value=True)
                    nc.scalar.activation(LNO[:], LNIN[:], Act.Ln)

                    # ---- masked accums ----
                    nc.vector.scalar_tensor_tensor(
                        DUM[:], CIO[:], 1.0, MSK[:], Alu.mult, Alu.mult,
                        accum_out=ACC[:, 4 * k:4 * k + 1])
                    nc.vector.scalar_tensor_tensor(
                        DUM[:], LNO[:, 0:W], 1.0, MSK[:], Alu.mult, Alu.mult,
                        accum_out=ACC[:, 4 * k + 1:4 * k + 2])
                    nc.vector.scalar_tensor_tensor(
                        DUM[:], LNO[:, W:2 * W], 1.0, MSK[:], Alu.mult,
                        Alu.mult, accum_out=ACC[:, 4 * k + 2:4 * k + 3])
                    nc.vector.tensor_scalar(
                        DUM[:], MSK[:], 1.0, 0.0, Alu.mult, Alu.add,
                        accum_out=ACC[:, 4 * k + 3:4 * k + 4])

            nc.sync.dma_start(accO, ACC[:])

    nc.compile()
    return nc


def kernel(pred, target):
    pred = np.ascontiguousarray(np.asarray(pred, dtype=np.float32))
    target = np.ascontiguousarray(np.asarray(target, dtype=np.float32))
    assert pred.shape == (B, A, N, CH) and target.shape == (B, N, CH)

    if "nc" not in _CACHE:
        _CACHE["nc"] = _build_bass()
    nc = _CACHE["nc"]

    from concourse import bass_utils

    in_maps = []
    for c in range(NCORES):
        lo, hi = c * BPC, (c + 1) * BPC
        in_maps.append({
            "predL": np.ascontiguousarray(pred[lo:hi]),
            "targL": np.ascontiguousarray(target[lo:hi]),
        })

    res = None
    for attempt in range(3):
        try:
            res = bass_utils.run_bass_kernel_spmd(
                nc, in_maps, core_ids=list(range(NCORES)))
            break
        except Exception:
            if attempt == 2:
                raise
    _CACHE["last_results"] = res

    per_batch = []
    for c in range(NCORES):
        acc = res.results[c]["acc_out"].astype(np.float32)
        acc = acc.reshape(PPART, N_CHUNKS, 4).sum(axis=1)
        num = acc[:, 0] - acc[:, 1] - 0.1 * acc[:, 2]
        cnt = acc[:, 3]
        nb = num.reshape(BPC, SEC).sum(axis=1, dtype=np.float32)
        cb = cnt.reshape(BPC, SEC).sum(axis=1, dtype=np.float32)
        per_batch.append(nb / cb)
    loss = np.mean(np.concatenate(per_batch), dtype=np.float32)
    return np.float32(loss)


# revision 5
# speedup vs baseline: 2.0731x; 1.1075x over previous
"""Trainium2 Bass kernel for nn_CustomLoss_87522843558003 (YOLO CIoU+BCE loss).

Strategy (data-parallel over batch, 8 cores):
 - Each core: 8 batches; positions map to 128 SBUF partitions as
   [batch(8) x section(16)] rows of 525 positions; j-axis split into chunks
   for DMA/compute overlap (double-buffered pools).
 - Loads: SWDGE (gpsimd) cast-DMA f32->bf16, one DMA per anchor + target
   per chunk; 16-bit data halves SBUF and enables DVE 2x modes.
 - Compute rebalanced off gpsimd: DVE does the elementwise work (planar
   component-major contiguous intermediates), ACT does copies/squares/Ln
   (one table set: natural_log has Square/Copy as fillers), POOL only a few
   independent strided 2-input products.
 - Per-anchor IoU: box corners + overlap; q = I * recip_fast(I+areas-I).
   Anchor argmax = first-max cascade with copy_predicated (15-channel
   selection); the selected q IS the CIoU iou (no recompute; eps diff 1e-7).
 - arctan pair eliminated: atan(rt)-atan(rp) = atan(u) with
   u = (wt*hp - wp*ht)/(hp*ht + wt*wp); (4/pi^2)*atan^2(sqrt(z)) fitted by
   rational F(z) = z(z+b)/(z^2+cz+d).
 - BCE: obj term = -ln(p_obj) at masked positions; cls term via
   sum_c ln|p+t-1-s| = ln prod_c |...| (one reduce, one Ln); small shift s
   keeps factors nonzero under bf16 rounding (ref clamps logs anyway).
 - Per-partition masked sums via accum_out -> [128, 4*n_chunks]; host sums
   chunks and normalizes per batch.
"""

import contextlib
import numpy as np

B, A, N, CH = 64, 3, 8400, 15
NCORES = 8
BPC = B // NCORES      # batches per core
SEC = 16               # partition sections per batch
PPART = BPC * SEC      # 128 partitions
W5 = N // SEC          # 525 positions per partition row
NCLS = 10
EPS = 1e-7
# rational fit of (4/pi^2)*atan(sqrt(z))^2
FB = 18.5807497
FC = 29.74781457
FD = 47.19260109

N_CHUNKS = 2
HALF = "bf16"

_CACHE = {}


def _build_bass(loop_r=None, n_chunks=N_CHUNKS, half=HALF, level=4,
                sbufs=2):
    import concourse.tile as tile
    import concourse.mybir as mybir
    from concourse import bacc

    Alu = mybir.AluOpType
    Act = mybir.ActivationFunctionType
    f32 = mybir.dt.float32
    i32 = mybir.dt.int32
    if half == "bf16":
        dtl, i16 = mybir.dt.bfloat16, mybir.dt.int16
    elif half == "fp16":
        dtl, i16 = mybir.dt.float16, mybir.dt.int16
    else:
        dtl, i16 = f32, i32
    cast = dtl != f32

    nc = bacc.Bacc("TRN2", target_bir_lowering=False, debug=False,
                   num_devices=NCORES)
    predL = nc.dram_tensor("predL", [BPC, A, N, CH], f32, kind="ExternalInput").ap()
    targL = nc.dram_tensor("targL", [BPC, N, CH], f32, kind="ExternalInput").ap()
    NACC = 4 * n_chunks
    accO = nc.dram_tensor("acc_out", [PPART, NACC], f32, kind="ExternalOutput").ap()

    base = W5 // n_chunks
    sizes = [base + (1 if i < W5 % n_chunks else 0) for i in range(n_chunks)]
    offs = [sum(sizes[:i]) for i in range(n_chunks)]

    pre = predL.rearrange("b a (s j) c -> b a s (j c)", s=SEC)
    tre = targL.rearrange("b (s j) c -> b s (j c)", s=SEC)

    with tile.TileContext(nc) as tc:
        with (
            tc.tile_pool(name="pIO", bufs=2) as pIO,
            tc.tile_pool(name="pS", bufs=sbufs) as pS,
            tc.tile_pool(name="pAcc", bufs=1) as pAcc,
        ):
            ACC = pAcc.tile([PPART, NACC], f32)
            loop_cm = tc.For_i(0, loop_r, 1) if loop_r else contextlib.nullcontext()
            with loop_cm:
                for k in range(n_chunks):
                    W, ofs = sizes[k], offs[k]
                    lo, hi = ofs * CH, (ofs + W) * CH

                    # ---- loads ----
                    T = pIO.tile([PPART, W * CH], dtl, name="T")
                    Ps = []
                    if cast:
                        nc.gpsimd.dma_start(T[:], tre[:, :, lo:hi])
                    else:
                        nc.scalar.dma_start(T[:], tre[:, :, lo:hi])
                    rings = [nc.sync, nc.scalar, nc.sync]
                    for a in range(A):
                        P = pIO.tile([PPART, W * CH], dtl, name=f"P{a}")
                        if cast:
                            nc.gpsimd.dma_start(P[:], pre[:, a][:, :, lo:hi])
                        else:
                            rings[a].dma_start(P[:], pre[:, a][:, :, lo:hi])
                        Ps.append(P)

                    DUM = pS.tile([PPART, W], f32, name="DUM")
                    if level == 0:
                        nc.vector.tensor_scalar(
                            DUM[:], T[:, 0:W], 1.0, 0.0, Alu.mult, Alu.add,
                            accum_out=ACC[:, 4 * k:4 * k + 1])
                        for a in range(A):
                            nc.vector.tensor_scalar(
                                DUM[:], Ps[a][:, 0:W], 1.0, 0.0, Alu.mult,
                                Alu.add,
                                accum_out=ACC[:, 4 * k + 1 + a:4 * k + 2 + a])
                        continue

                    Tcj = T[:].rearrange("p (j c) -> p c j", c=CH)
                    Tr = T[:].rearrange("p (j c) -> p j c", c=CH)

                    # ---- target prep ----
                    TLO = pS.tile([PPART, 2 * W], dtl, name="TLO")
                    THI = pS.tile([PPART, 2 * W], dtl, name="THI")
                    TA = pS.tile([PPART, W], dtl, name="TA")
                    tlov = TLO[:].rearrange("p (c j) -> p c j", c=2)
                    thiv = THI[:].rearrange("p (c j) -> p c j", c=2)
                    nc.vector.scalar_tensor_tensor(
                        tlov, Tcj[:, 2:4], -0.5, Tcj[:, 0:2], Alu.mult, Alu.add)
                    nc.vector.scalar_tensor_tensor(
                        thiv, Tcj[:, 2:4], 0.5, Tcj[:, 0:2], Alu.mult, Alu.add)
                    nc.gpsimd.tensor_tensor(TA[:], Tcj[:, 2], Tcj[:, 3], Alu.mult)
                    MSK = Tcj[:, 4]

                    # ---- per-anchor IoU (I, U, q = I/U) ----
                    Qs = []
                    for a in range(A):
                        Pcj = Ps[a][:].rearrange("p (j c) -> p c j", c=CH)
                        WHH = pS.tile([PPART, 2 * W], dtl, name=f"WHH{a}")
                        LO = pS.tile([PPART, 2 * W], dtl, name=f"LO{a}")
                        HI = pS.tile([PPART, 2 * W], dtl, name=f"HI{a}")
                        WD = pS.tile([PPART, 2 * W], dtl, name=f"WD{a}")
                        whhv = WHH[:].rearrange("p (c j) -> p c j", c=2)
                        lov = LO[:].rearrange("p (c j) -> p c j", c=2)
                        hiv = HI[:].rearrange("p (c j) -> p c j", c=2)
                        nc.scalar.activation(whhv, Pcj[:, 2:4], Act.Copy,
                                             scale=0.5)
                        nc.vector.tensor_tensor(lov, Pcj[:, 0:2], whhv,
                                                Alu.subtract)
                        nc.vector.tensor_tensor(hiv, Pcj[:, 0:2], whhv,
                                                Alu.add)
                        nc.vector.tensor_tensor(WHH[:], LO[:], TLO[:], Alu.max)
                        nc.vector.tensor_tensor(WD[:], HI[:], THI[:], Alu.min)
                        nc.vector.tensor_tensor(WD[:], WD[:], WHH[:],
                                                Alu.subtract)
                        nc.vector.tensor_scalar(WD[:], WD[:], 0.0, None, Alu.max)
                        IA = pS.tile([PPART, W], dtl, name=f"IA{a}")
                        PAr = pS.tile([PPART, W], dtl, name=f"PAr{a}")
                        UA = pS.tile([PPART, W], f32, name=f"UA{a}")
                        QA = pS.tile([PPART, W], f32, name=f"QA{a}")
                        nc.vector.tensor_tensor(IA[:], WD[:, 0:W], WD[:, W:],
                                                Alu.mult)
                        nc.gpsimd.tensor_tensor(PAr[:], Pcj[:, 2], Pcj[:, 3],
                                                Alu.mult)
                        nc.vector.tensor_tensor(UA[:], PAr[:], TA[:], Alu.add)
                        nc.vector.tensor_tensor(UA[:], UA[:], IA[:],
                                                Alu.subtract)
                        nc.vector.reciprocal_approx_fast(QA[:], UA[:])
                        nc.vector.tensor_tensor(QA[:], IA[:], QA[:], Alu.mult)
                        Qs.append(QA)
                        if level == 1:
                            nc.vector.tensor_scalar(
                                DUM[:], QA[:], 1.0, 0.0, Alu.mult, Alu.add,
                                accum_out=ACC[:, 4 * k + a:4 * k + a + 1])
                    if level == 1:
                        continue

                    # ---- selection cascade (first-max argmax) ----
                    SELP = Ps[0]
                    SELQ = Qs[0]
                    for a in (1, 2):
                        G = pS.tile([PPART, W], f32, name=f"G{a}")
                        nc.vector.tensor_tensor(G[:], SELQ[:], Qs[a][:],
                                                Alu.is_ge)
                        nc.vector.copy_predicated(
                            Qs[a][:], G[:].bitcast(i32), SELQ[:])
                        if cast:
                            GH = pS.tile([PPART, W], dtl, name=f"GH{a}")
                            nc.vector.tensor_copy(GH[:], G[:])
                            mask = GH[:].bitcast(i16).unsqueeze(2).broadcast_to(
                                [PPART, W, CH])
                        else:
                            mask = G[:].bitcast(i32).unsqueeze(2).broadcast_to(
                                [PPART, W, CH])
                        nc.vector.copy_predicated(
                            Ps[a][:].rearrange("p (j c) -> p j c", c=CH), mask,
                            SELP[:].rearrange("p (j c) -> p j c", c=CH))
                        SELP = Ps[a]
                        SELQ = Qs[a]
                    IOU = SELQ  # f32 [P, W] == selected inter/union
                    if level == 2:
                        nc.vector.tensor_scalar(
                            DUM[:], IOU[:], 1.0, 0.0, Alu.mult, Alu.add,
                            accum_out=ACC[:, 4 * k:4 * k + 1])
                        nc.vector.tensor_scalar(
                            DUM[:], SELP[:, 0:W], 1.0, 0.0, Alu.mult, Alu.add,
                            accum_out=ACC[:, 4 * k + 1:4 * k + 2])
                        continue

                    Scj = SELP[:].rearrange("p (j c) -> p c j", c=CH)
                    Sr = SELP[:].rearrange("p (j c) -> p j c", c=CH)

                    # ---- CIoU on selected ----
                    SWH = pS.tile([PPART, 2 * W], dtl, name="SWH")
                    SLO = pS.tile([PPART, 2 * W], dtl, name="SLO")
                    SHI = pS.tile([PPART, 2 * W], dtl, name="SHI")
                    CW = pS.tile([PPART, 2 * W], dtl, name="CW")
                    CW2 = pS.tile([PPART, 2 * W], f32, name="CW2")
                    swhv = SWH[:].rearrange("p (c j) -> p c j", c=2)
                    slov = SLO[:].rearrange("p (c j) -> p c j", c=2)
                    shiv = SHI[:].rearrange("p (c j) -> p c j", c=2)
                    nc.scalar.activation(swhv, Scj[:, 2:4], Act.Copy, scale=0.5)
                    nc.vector.tensor_tensor(slov, Scj[:, 0:2], swhv,
                                            Alu.subtract)
                    nc.vector.tensor_tensor(shiv, Scj[:, 0:2], swhv, Alu.add)
                    nc.vector.tensor_tensor(SLO[:], SLO[:], TLO[:], Alu.min)
                    nc.vector.tensor_tensor(SHI[:], SHI[:], THI[:], Alu.max)
                    nc.vector.tensor_tensor(CW[:], SHI[:], SLO[:], Alu.subtract)
                    nc.scalar.activation(CW2[:], CW[:], Act.Square)
                    DIAG = pS.tile([PPART, W], f32, name="DIAG")
                    RDG = pS.tile([PPART, W], f32, name="RDG")
                    nc.vector.scalar_tensor_tensor(
                        DIAG[:], CW2[:, 0:W], EPS, CW2[:, W:], Alu.add, Alu.add)
                    nc.vector.reciprocal_approx_fast(RDG[:], DIAG[:])
                    DXY = pS.tile([PPART, 2 * W], dtl, name="DXY")
                    DXY2 = pS.tile([PPART, 2 * W], f32, name="DXY2")
                    nc.vector.tensor_tensor(
                        DXY[:].rearrange("p (c j) -> p c j", c=2),
                        Scj[:, 0:2], Tcj[:, 0:2], Alu.subtract)
                    nc.scalar.activation(DXY2[:], DXY[:], Act.Square)
                    CD = pS.tile([PPART, W], f32, name="CD")
                    QD = pS.tile([PPART, W], f32, name="QD")
                    OMIE = pS.tile([PPART, W], f32, name="OMIE")
                    DIOU = pS.tile([PPART, W], f32, name="DIOU")
                    nc.gpsimd.tensor_tensor(CD[:], DXY2[:, 0:W], DXY2[:, W:],
                                            Alu.add)
                    nc.vector.tensor_tensor(QD[:], CD[:], RDG[:], Alu.mult)
                    nc.vector.tensor_scalar(OMIE[:], IOU[:], -1.0, 1.0 + EPS,
                                            Alu.mult, Alu.add)
                    nc.gpsimd.tensor_tensor(DIOU[:], OMIE[:], QD[:], Alu.add)

                    # v-term
                    N1 = pS.tile([PPART, W], dtl, name="N1")
                    N2 = pS.tile([PPART, W], dtl, name="N2")
                    D1 = pS.tile([PPART, W], dtl, name="D1")
                    D2 = pS.tile([PPART, W], dtl, name="D2")
                    nc.gpsimd.tensor_tensor(N1[:], Tcj[:, 2], Scj[:, 3], Alu.mult)
                    nc.gpsimd.tensor_tensor(N2[:], Scj[:, 2], Tcj[:, 3], Alu.mult)
                    nc.gpsimd.tensor_tensor(D1[:], Scj[:, 3], Tcj[:, 3], Alu.mult)
                    nc.gpsimd.tensor_tensor(D2[:], Scj[:, 2], Tcj[:, 2], Alu.mult)
                    NUM = pS.tile([PPART, W], f32, name="NUM")
                    DEN = pS.tile([PPART, W], f32, name="DEN")
                    UU = pS.tile([PPART, W], f32, name="UU")
                    ZZ = pS.tile([PPART, W], f32, name="ZZ")
                    VN = pS.tile([PPART, W], f32, name="VN")
                    VD = pS.tile([PPART, W], f32, name="VD")
                    VV = pS.tile([PPART, W], f32, name="VV")
                    AD = pS.tile([PPART, W], f32, name="AD")
                    CIO = pS.tile([PPART, W], f32, name="CIO")
                    nc.gpsimd.tensor_tensor(NUM[:], N1[:], N2[:], Alu.subtract)
                    nc.gpsimd.tensor_tensor(DEN[:], D1[:], D2[:], Alu.add)
                    nc.vector.reciprocal_approx_fast(UU[:], DEN[:])
                    nc.vector.tensor_tensor(UU[:], NUM[:], UU[:], Alu.mult)
                    nc.scalar.activation(ZZ[:], UU[:], Act.Square)
                    nc.vector.scalar_tensor_tensor(VN[:], ZZ[:], FB, ZZ[:],
                                                   Alu.add, Alu.mult)
                    nc.vector.scalar_tensor_tensor(VD[:], ZZ[:], FC, ZZ[:],
                                                   Alu.add, Alu.mult)
                    nc.vector.tensor_scalar(VD[:], VD[:], 1.0, FD, Alu.mult,
                                            Alu.add)
                    nc.vector.reciprocal_approx_fast(VV[:], VD[:])
                    nc.vector.tensor_tensor(VV[:], VN[:], VV[:], Alu.mult)
                    nc.vector.tensor_tensor(AD[:], VV[:], OMIE[:], Alu.add)
                    nc.vector.reciprocal_approx_fast(AD[:], AD[:])
                    nc.scalar.activation(VN[:], VV[:], Act.Square)
                    nc.vector.tensor_tensor(AD[:], VN[:], AD[:], Alu.mult)
                    nc.vector.tensor_tensor(CIO[:], DIOU[:], AD[:], Alu.add)
                    if level == 3:
                        nc.vector.scalar_tensor_tensor(
                            DUM[:], CIO[:], 1.0, MSK, Alu.mult, Alu.mult,
                            accum_out=ACC[:, 4 * k:4 * k + 1])
                        nc.vector.tensor_scalar(
                            DUM[:], MSK, 1.0, 0.0, Alu.mult, Alu.add,
                            accum_out=ACC[:, 4 * k + 3:4 * k + 4])
                        continue

                    # ---- BCE prep ----
                    LNIN = pS.tile([PPART, W], f32, name="LNIN")
                    LNO = pS.tile([PPART, 2 * W], f32, name="LNO")
                    DT = pS.tile([PPART, W * NCLS], dtl, name="DT")
                    nc.scalar.activation(LNO[:, 0:W], Scj[:, 4], Act.Ln)
                    # shift keeps |p+t-1-shift| > 0 under 16-bit rounding
                    # (ref clamps logs at -100 anyway); bias ~4e-3 per term
                    shift = -1.0005 if cast else -1.0
                    nc.vector.scalar_tensor_tensor(
                        DT[:].rearrange("p (j c) -> p j c", c=NCLS),
                        Sr[:, :, 5:CH], shift, Tr[:, :, 5:CH], Alu.add, Alu.add)
                    nc.vector.tensor_reduce(
                        LNIN[:],
                        DT[:].rearrange("p (j c) -> p j c", c=NCLS),
                        mybir.AxisListType.X, Alu.mult,
                        apply_absolute_value=True)
                    nc.scalar.activation(LNO[:, W:2 * W], LNIN[:], Act.Ln)

                    # ---- masked accums ----
                    nc.vector.scalar_tensor_tensor(
                        DUM[:], CIO[:], 1.0, MSK, Alu.mult, Alu.mult,
                        accum_out=ACC[:, 4 * k:4 * k + 1])
                    nc.vector.scalar_tensor_tensor(
                        DUM[:], LNO[:, 0:W], 1.0, MSK, Alu.mult, Alu.mult,
                        accum_out=ACC[:, 4 * k + 1:4 * k + 2])
                    nc.vector.scalar_tensor_tensor(
                        DUM[:], LNO[:, W:2 * W], 1.0, MSK, Alu.mult,
                        Alu.mult, accum_out=ACC[:, 4 * k + 2:4 * k + 3])
                    nc.vector.tensor_scalar(
                        DUM[:], MSK, 1.0, 0.0, Alu.mult, Alu.add,
                        accum_out=ACC[:, 4 * k + 3:4 * k + 4])

            nc.sync.dma_start(accO, ACC[:])

    nc.compile()
    return nc


def kernel(pred, target):
    pred = np.ascontiguousarray(np.asarray(pred, dtype=np.float32))
    target = np.ascontiguousarray(np.asarray(target, dtype=np.float32))
    assert pred.shape == (B, A, N, CH) and target.shape == (B, N, CH)

    if "nc" not in _CACHE:
        _CACHE["nc"] = _build_bass()
    nc = _CACHE["nc"]

    from concourse import bass_utils

    in_maps = []
    for c in range(NCORES):
        lo, hi = c * BPC, (c + 1) * BPC
        in_maps.append({
            "predL": np.ascontiguousarray(pred[lo:hi]),
            "targL": np.ascontiguousarray(target[lo:hi]),
        })

    res = None
    for attempt in range(3):
        try:
            res = bass_utils.run_bass_kernel_spmd(
                nc, in_maps, core_ids=list(range(NCORES)))
            break
        except Exception:
            if attempt == 2:
                raise
    _CACHE["last_results"] = res

    per_batch = []
    for c in range(NCORES):
        acc = res.results[c]["acc_out"].astype(np.float32)
        acc = acc.reshape(PPART, N_CHUNKS, 4).sum(axis=1)
        num = acc[:, 0] - acc[:, 1] - 0.1 * acc[:, 2]
        cnt = acc[:, 3]
        nb = num.reshape(BPC, SEC).sum(axis=1, dtype=np.float32)
        cb = cnt.reshape(BPC, SEC).sum(axis=1, dtype=np.float32)
        per_batch.append(nb / cb)
    loss = np.mean(np.concatenate(per_batch), dtype=np.float32)
    return np.float32(loss)


# revision 6
# speedup vs baseline: 2.1038x; 1.0148x over previous
"""Trainium2 Bass kernel for nn_CustomLoss_87522843558003 (YOLO CIoU+BCE loss).

Strategy (data-parallel over batch, 8 cores):
 - Each core: 8 batches; positions map to 128 SBUF partitions as
   [batch(8) x section(16)] rows of 525 positions; j-axis split into chunks
   for DMA/compute overlap (double-buffered pools).
 - Loads: SWDGE (gpsimd) cast-DMA f32->bf16, one DMA per anchor + target
   per chunk; 16-bit data halves SBUF and enables DVE 2x modes.
 - Compute rebalanced off gpsimd: DVE does the elementwise work (planar
   component-major contiguous intermediates), ACT does copies/squares/Ln
   (one table set: natural_log has Square/Copy as fillers), POOL only a few
   independent strided 2-input products.
 - Per-anchor IoU: box corners + overlap; q = I * recip_fast(I+areas-I).
   Anchor argmax = first-max cascade with copy_predicated (15-channel
   selection); the selected q IS the CIoU iou (no recompute; eps diff 1e-7).
 - arctan pair eliminated: atan(rt)-atan(rp) = atan(u) with
   u = (wt*hp - wp*ht)/(hp*ht + wt*wp); (4/pi^2)*atan^2(sqrt(z)) fitted by
   rational F(z) = z(z+b)/(z^2+cz+d).
 - BCE: obj term = -ln(p_obj) at masked positions; cls term via
   sum_c ln|p+t-1-s| = ln prod_c |...| (one reduce, one Ln); small shift s
   keeps factors nonzero under bf16 rounding (ref clamps logs anyway).
 - Per-partition masked sums via accum_out -> [128, 4*n_chunks]; host sums
   chunks and normalizes per batch.
"""

import contextlib
import numpy as np

B, A, N, CH = 64, 3, 8400, 15
NCORES = 8
BPC = B // NCORES      # batches per core
SEC = 16               # partition sections per batch
PPART = BPC * SEC      # 128 partitions
W5 = N // SEC          # 525 positions per partition row
NCLS = 10
EPS = 1e-7
# rational fit of (4/pi^2)*atan(sqrt(z))^2
FB = 18.5807497
FC = 29.74781457
FD = 47.19260109

N_CHUNKS = 2
HALF = "bf16"

_CACHE = {}


def _build_bass(loop_r=None, n_chunks=N_CHUNKS, half=HALF, level=4,
                sbufs=2):
    import concourse.tile as tile
    import concourse.mybir as mybir
    from concourse import bacc

    Alu = mybir.AluOpType
    Act = mybir.ActivationFunctionType
    f32 = mybir.dt.float32
    i32 = mybir.dt.int32
    if half == "bf16":
        dtl, i16 = mybir.dt.bfloat16, mybir.dt.int16
    elif half == "fp16":
        dtl, i16 = mybir.dt.float16, mybir.dt.int16
    else:
        dtl, i16 = f32, i32
    cast = dtl != f32

    nc = bacc.Bacc("TRN2", target_bir_lowering=False, debug=False,
                   num_devices=NCORES)
    predL = nc.dram_tensor("predL", [BPC, A, N, CH], f32, kind="ExternalInput").ap()
    targL = nc.dram_tensor("targL", [BPC, N, CH], f32, kind="ExternalInput").ap()
    NACC = 4 * n_chunks
    accO = nc.dram_tensor("acc_out", [PPART, NACC], f32, kind="ExternalOutput").ap()

    base = W5 // n_chunks
    sizes = [base + (1 if i < W5 % n_chunks else 0) for i in range(n_chunks)]
    offs = [sum(sizes[:i]) for i in range(n_chunks)]

    pre = predL.rearrange("b a (s j) c -> b a s (j c)", s=SEC)
    tre = targL.rearrange("b (s j) c -> b s (j c)", s=SEC)

    with tile.TileContext(nc) as tc:
        with (
            tc.tile_pool(name="pIO", bufs=2) as pIO,
            tc.tile_pool(name="pS", bufs=sbufs) as pS,
            tc.tile_pool(name="pAcc", bufs=1) as pAcc,
        ):
            ACC = pAcc.tile([PPART, NACC], f32)
            loop_cm = (tc.For_i(0, loop_r, 1, staggered_reset=True)
                       if loop_r else contextlib.nullcontext())
            with loop_cm:
                for k in range(n_chunks):
                    W, ofs = sizes[k], offs[k]
                    lo, hi = ofs * CH, (ofs + W) * CH

                    # ---- loads ----
                    T = pIO.tile([PPART, W * CH], dtl, name="T")
                    Ps = []
                    if cast:
                        nc.gpsimd.dma_start(T[:], tre[:, :, lo:hi])
                    else:
                        nc.scalar.dma_start(T[:], tre[:, :, lo:hi])
                    rings = [nc.sync, nc.scalar, nc.sync]
                    for a in range(A):
                        P = pIO.tile([PPART, W * CH], dtl, name=f"P{a}")
                        if cast:
                            nc.gpsimd.dma_start(P[:], pre[:, a][:, :, lo:hi])
                        else:
                            rings[a].dma_start(P[:], pre[:, a][:, :, lo:hi])
                        Ps.append(P)

                    DUM = pS.tile([PPART, W], f32, name="DUM")
                    if level == 0:
                        nc.vector.tensor_scalar(
                            DUM[:], T[:, 0:W], 1.0, 0.0, Alu.mult, Alu.add,
                            accum_out=ACC[:, 4 * k:4 * k + 1])
                        for a in range(A):
                            nc.vector.tensor_scalar(
                                DUM[:], Ps[a][:, 0:W], 1.0, 0.0, Alu.mult,
                                Alu.add,
                                accum_out=ACC[:, 4 * k + 1 + a:4 * k + 2 + a])
                        continue

                    Tcj = T[:].rearrange("p (j c) -> p c j", c=CH)
                    Tr = T[:].rearrange("p (j c) -> p j c", c=CH)

                    # ---- target prep ----
                    TLO = pS.tile([PPART, 2 * W], dtl, name="TLO")
                    THI = pS.tile([PPART, 2 * W], dtl, name="THI")
                    TA = pS.tile([PPART, W], dtl, name="TA")
                    tlov = TLO[:].rearrange("p (c j) -> p c j", c=2)
                    thiv = THI[:].rearrange("p (c j) -> p c j", c=2)
                    nc.vector.scalar_tensor_tensor(
                        tlov, Tcj[:, 2:4], -0.5, Tcj[:, 0:2], Alu.mult, Alu.add)
                    nc.vector.scalar_tensor_tensor(
                        thiv, Tcj[:, 2:4], 0.5, Tcj[:, 0:2], Alu.mult, Alu.add)
                    nc.gpsimd.tensor_tensor(TA[:], Tcj[:, 2], Tcj[:, 3], Alu.mult)
                    MSK = Tcj[:, 4]

                    # ---- per-anchor IoU (I, U, q = I/U) ----
                    Qs = []
                    for a in range(A):
                        Pcj = Ps[a][:].rearrange("p (j c) -> p c j", c=CH)
                        WHH = pS.tile([PPART, 2 * W], dtl, name=f"WHH{a}")
                        LO = pS.tile([PPART, 2 * W], dtl, name=f"LO{a}")
                        HI = pS.tile([PPART, 2 * W], dtl, name=f"HI{a}")
                        WD = pS.tile([PPART, 2 * W], dtl, name=f"WD{a}")
                        whhv = WHH[:].rearrange("p (c j) -> p c j", c=2)
                        lov = LO[:].rearrange("p (c j) -> p c j", c=2)
                        hiv = HI[:].rearrange("p (c j) -> p c j", c=2)
                        nc.scalar.activation(whhv, Pcj[:, 2:4], Act.Copy,
                                             scale=0.5)
                        nc.vector.tensor_tensor(lov, Pcj[:, 0:2], whhv,
                                                Alu.subtract)
                        nc.vector.tensor_tensor(hiv, Pcj[:, 0:2], whhv,
                                                Alu.add)
                        nc.vector.tensor_tensor(WHH[:], LO[:], TLO[:], Alu.max)
                        nc.vector.tensor_tensor(WD[:], HI[:], THI[:], Alu.min)
                        nc.vector.tensor_tensor(WD[:], WD[:], WHH[:],
                                                Alu.subtract)
                        nc.vector.tensor_scalar(WD[:], WD[:], 0.0, None, Alu.max)
                        IA = pS.tile([PPART, W], dtl, name=f"IA{a}")
                        PAr = pS.tile([PPART, W], dtl, name=f"PAr{a}")
                        UA = pS.tile([PPART, W], f32, name=f"UA{a}")
                        QA = pS.tile([PPART, W], f32, name=f"QA{a}")
                        nc.vector.tensor_tensor(IA[:], WD[:, 0:W], WD[:, W:],
                                                Alu.mult)
                        nc.gpsimd.tensor_tensor(PAr[:], Pcj[:, 2], Pcj[:, 3],
                                                Alu.mult)
                        nc.vector.tensor_tensor(UA[:], PAr[:], TA[:], Alu.add)
                        nc.vector.tensor_tensor(UA[:], UA[:], IA[:],
                                                Alu.subtract)
                        nc.vector.reciprocal_approx_fast(QA[:], UA[:])
                        nc.vector.tensor_tensor(QA[:], IA[:], QA[:], Alu.mult)
                        Qs.append(QA)
                        if level == 1:
                            nc.vector.tensor_scalar(
                                DUM[:], QA[:], 1.0, 0.0, Alu.mult, Alu.add,
                                accum_out=ACC[:, 4 * k + a:4 * k + a + 1])
                    if level == 1:
                        continue

                    # ---- selection cascade (first-max argmax) ----
                    SELP = Ps[0]
                    SELQ = Qs[0]
                    for a in (1, 2):
                        G = pS.tile([PPART, W], f32, name=f"G{a}")
                        nc.vector.tensor_tensor(G[:], SELQ[:], Qs[a][:],
                                                Alu.is_ge)
                        nc.vector.copy_predicated(
                            Qs[a][:], G[:].bitcast(i32), SELQ[:])
                        if cast:
                            GH = pS.tile([PPART, W], dtl, name=f"GH{a}")
                            nc.vector.tensor_copy(GH[:], G[:])
                            mask = GH[:].bitcast(i16).unsqueeze(2).broadcast_to(
                                [PPART, W, CH])
                        else:
                            mask = G[:].bitcast(i32).unsqueeze(2).broadcast_to(
                                [PPART, W, CH])
                        nc.vector.copy_predicated(
                            Ps[a][:].rearrange("p (j c) -> p j c", c=CH), mask,
                            SELP[:].rearrange("p (j c) -> p j c", c=CH))
                        SELP = Ps[a]
                        SELQ = Qs[a]
                    IOU = SELQ  # f32 [P, W] == selected inter/union
                    if level == 2:
                        nc.vector.tensor_scalar(
                            DUM[:], IOU[:], 1.0, 0.0, Alu.mult, Alu.add,
                            accum_out=ACC[:, 4 * k:4 * k + 1])
                        nc.vector.tensor_scalar(
                            DUM[:], SELP[:, 0:W], 1.0, 0.0, Alu.mult, Alu.add,
                            accum_out=ACC[:, 4 * k + 1:4 * k + 2])
                        continue

                    Scj = SELP[:].rearrange("p (j c) -> p c j", c=CH)
                    Sr = SELP[:].rearrange("p (j c) -> p j c", c=CH)

                    # ---- CIoU on selected ----
                    SWH = pS.tile([PPART, 2 * W], dtl, name="SWH")
                    SLO = pS.tile([PPART, 2 * W], dtl, name="SLO")
                    SHI = pS.tile([PPART, 2 * W], dtl, name="SHI")
                    CW = pS.tile([PPART, 2 * W], dtl, name="CW")
                    CW2 = pS.tile([PPART, 2 * W], f32, name="CW2")
                    swhv = SWH[:].rearrange("p (c j) -> p c j", c=2)
                    slov = SLO[:].rearrange("p (c j) -> p c j", c=2)
                    shiv = SHI[:].rearrange("p (c j) -> p c j", c=2)
                    nc.scalar.activation(swhv, Scj[:, 2:4], Act.Copy, scale=0.5)
                    nc.vector.tensor_tensor(slov, Scj[:, 0:2], swhv,
                                            Alu.subtract)
                    nc.vector.tensor_tensor(shiv, Scj[:, 0:2], swhv, Alu.add)
                    nc.vector.tensor_tensor(SLO[:], SLO[:], TLO[:], Alu.min)
                    nc.vector.tensor_tensor(SHI[:], SHI[:], THI[:], Alu.max)
                    nc.vector.tensor_tensor(CW[:], SHI[:], SLO[:], Alu.subtract)
                    nc.scalar.activation(CW2[:], CW[:], Act.Square)
                    DIAG = pS.tile([PPART, W], f32, name="DIAG")
                    RDG = pS.tile([PPART, W], f32, name="RDG")
                    nc.vector.scalar_tensor_tensor(
                        DIAG[:], CW2[:, 0:W], EPS, CW2[:, W:], Alu.add, Alu.add)
                    nc.vector.reciprocal_approx_fast(RDG[:], DIAG[:])
                    DXY = pS.tile([PPART, 2 * W], dtl, name="DXY")
                    DXY2 = pS.tile([PPART, 2 * W], f32, name="DXY2")
                    nc.vector.tensor_tensor(
                        DXY[:].rearrange("p (c j) -> p c j", c=2),
                        Scj[:, 0:2], Tcj[:, 0:2], Alu.subtract)
                    nc.scalar.activation(DXY2[:], DXY[:], Act.Square)
                    CD = pS.tile([PPART, W], f32, name="CD")
                    QD = pS.tile([PPART, W], f32, name="QD")
                    OMIE = pS.tile([PPART, W], f32, name="OMIE")
                    DIOU = pS.tile([PPART, W], f32, name="DIOU")
                    nc.gpsimd.tensor_tensor(CD[:], DXY2[:, 0:W], DXY2[:, W:],
                                            Alu.add)
                    nc.vector.tensor_tensor(QD[:], CD[:], RDG[:], Alu.mult)
                    nc.vector.tensor_scalar(OMIE[:], IOU[:], -1.0, 1.0 + EPS,
                                            Alu.mult, Alu.add)
                    nc.gpsimd.tensor_tensor(DIOU[:], OMIE[:], QD[:], Alu.add)

                    # v-term
                    N1 = pS.tile([PPART, W], dtl, name="N1")
                    N2 = pS.tile([PPART, W], dtl, name="N2")
                    D1 = pS.tile([PPART, W], dtl, name="D1")
                    D2 = pS.tile([PPART, W], dtl, name="D2")
                    nc.gpsimd.tensor_tensor(N1[:], Tcj[:, 2], Scj[:, 3], Alu.mult)
                    nc.gpsimd.tensor_tensor(N2[:], Scj[:, 2], Tcj[:, 3], Alu.mult)
                    nc.gpsimd.tensor_tensor(D1[:], Scj[:, 3], Tcj[:, 3], Alu.mult)
                    nc.gpsimd.tensor_tensor(D2[:], Scj[:, 2], Tcj[:, 2], Alu.mult)
                    NUM = pS.tile([PPART, W], f32, name="NUM")
                    DEN = pS.tile([PPART, W], f32, name="DEN")
                    UU = pS.tile([PPART, W], f32, name="UU")
                    ZZ = pS.tile([PPART, W], f32, name="ZZ")
                    VN = pS.tile([PPART, W], f32, name="VN")
                    VD = pS.tile([PPART, W], f32, name="VD")
                    VV = pS.tile([PPART, W], f32, name="VV")
                    AD = pS.tile([PPART, W], f32, name="AD")
                    CIO = pS.tile([PPART, W], f32, name="CIO")
                    nc.gpsimd.tensor_tensor(NUM[:], N1[:], N2[:], Alu.subtract)
                    nc.gpsimd.tensor_tensor(DEN[:], D1[:], D2[:], Alu.add)
                    nc.vector.reciprocal_approx_fast(UU[:], DEN[:])
                    nc.vector.tensor_tensor(UU[:], NUM[:], UU[:], Alu.mult)
                    nc.scalar.activation(ZZ[:], UU[:], Act.Square)
                    nc.vector.scalar_tensor_tensor(VN[:], ZZ[:], FB, ZZ[:],
                                                   Alu.add, Alu.mult)
                    nc.vector.scalar_tensor_tensor(VD[:], ZZ[:], FC, ZZ[:],
                                                   Alu.add, Alu.mult)
                    nc.vector.tensor_scalar(VD[:], VD[:], 1.0, FD, Alu.mult,
                                            Alu.add)
                    nc.vector.reciprocal_approx_fast(VV[:], VD[:])
                    nc.vector.tensor_tensor(VV[:], VN[:], VV[:], Alu.mult)
                    nc.vector.tensor_tensor(AD[:], VV[:], OMIE[:], Alu.add)
                    nc.vector.reciprocal_approx_fast(AD[:], AD[:])
                    nc.scalar.activation(VN[:], VV[:], Act.Square)
                    nc.vector.tensor_tensor(AD[:], VN[:], AD[:], Alu.mult)
                    nc.vector.tensor_tensor(CIO[:], DIOU[:], AD[:], Alu.add)
                    if level == 3:
                        nc.vector.scalar_tensor_tensor(
                            DUM[:], CIO[:], 1.0, MSK, Alu.mult, Alu.mult,
                            accum_out=ACC[:, 4 * k:4 * k + 1])
                        nc.vector.tensor_scalar(
                            DUM[:], MSK, 1.0, 0.0, Alu.mult, Alu.add,
                            accum_out=ACC[:, 4 * k + 3:4 * k + 4])
                        continue

                    # ---- BCE prep ----
                    LNIN = pS.tile([PPART, W], f32, name="LNIN")
                    LNO = pS.tile([PPART, 2 * W], f32, name="LNO")
                    DT = pS.tile([PPART, W * NCLS], dtl, name="DT")
                    nc.scalar.activation(LNO[:, 0:W], Scj[:, 4], Act.Ln)
                    # shift keeps |p+t-1-shift| > 0 under 16-bit rounding
                    # (ref clamps logs at -100 anyway); bias ~4e-3 per term
                    shift = -1.0005 if cast else -1.0
                    nc.vector.scalar_tensor_tensor(
                        DT[:].rearrange("p (j c) -> p j c", c=NCLS),
                        Sr[:, :, 5:CH], shift, Tr[:, :, 5:CH], Alu.add, Alu.add)
                    nc.vector.tensor_reduce(
                        LNIN[:],
                        DT[:].rearrange("p (j c) -> p j c", c=NCLS),
                        mybir.AxisListType.X, Alu.mult,
                        apply_absolute_value=True)
                    nc.scalar.activation(LNO[:, W:2 * W], LNIN[:], Act.Ln)

                    # ---- masked accums ----
                    nc.vector.scalar_tensor_tensor(
                        DUM[:], CIO[:], 1.0, MSK, Alu.mult, Alu.mult,
                        accum_out=ACC[:, 4 * k:4 * k + 1])
                    nc.vector.scalar_tensor_tensor(
                        DUM[:], LNO[:, 0:W], 1.0, MSK, Alu.mult, Alu.mult,
                        accum_out=ACC[:, 4 * k + 1:4 * k + 2])
                    nc.vector.scalar_tensor_tensor(
                        DUM[:], LNO[:, W:2 * W], 1.0, MSK, Alu.mult,
                        Alu.mult, accum_out=ACC[:, 4 * k + 2:4 * k + 3])
                    nc.vector.tensor_scalar(
                        DUM[:], MSK, 1.0, 0.0, Alu.mult, Alu.add,
                        accum_out=ACC[:, 4 * k + 3:4 * k + 4])

            nc.sync.dma_start(accO, ACC[:])

    nc.compile()
    return nc


def kernel(pred, target):
    pred = np.ascontiguousarray(np.asarray(pred, dtype=np.float32))
    target = np.ascontiguousarray(np.asarray(target, dtype=np.float32))
    assert pred.shape == (B, A, N, CH) and target.shape == (B, N, CH)

    if "nc" not in _CACHE:
        _CACHE["nc"] = _build_bass()
    nc = _CACHE["nc"]

    from concourse import bass_utils

    in_maps = []
    for c in range(NCORES):
        lo, hi = c * BPC, (c + 1) * BPC
        in_maps.append({
            "predL": np.ascontiguousarray(pred[lo:hi]),
            "targL": np.ascontiguousarray(target[lo:hi]),
        })

    res = None
    for attempt in range(3):
        try:
            res = bass_utils.run_bass_kernel_spmd(
                nc, in_maps, core_ids=list(range(NCORES)))
            break
        except Exception:
            if attempt == 2:
                raise
    _CACHE["last_results"] = res

    per_batch = []
    for c in range(NCORES):
        acc = res.results[c]["acc_out"].astype(np.float32)
        acc = acc.reshape(PPART, N_CHUNKS, 4).sum(axis=1)
        num = acc[:, 0] - acc[:, 1] - 0.1 * acc[:, 2]
        cnt = acc[:, 3]
        nb = num.reshape(BPC, SEC).sum(axis=1, dtype=np.float32)
        cb = cnt.reshape(BPC, SEC).sum(axis=1, dtype=np.float32)
        per_batch.append(nb / cb)
    loss = np.mean(np.concatenate(per_batch), dtype=np.float32)
    return np.float32(loss)
